# revision 14
# baseline (speedup 1.0000x reference)
"""Two-layer GCN (PyG GCNConv semantics) on 8 Trainium2 NeuronCores.

Strategy: partition nodes (and their incident edges, by dst) across the 8
cores; each core aggregates messages for its own dst nodes with dma_gather
(random row gather from a replicated table) + one-hot selection-matrix
matmuls that perform the segment-sum in PSUM.

The symmetric norm dinv[src]*dinv[dst] is factorized: dinv[src] is folded
into the table rows (host-side for x, an Activation-engine scale for h2)
and dinv[dst] into the per-block epilogue PSUM->SBUF copy, so the selection
matrices are pure 0/1 one-hots and the self-loop diagonal is a constant
identity matmul.

Layer 1 gathers dinv-scaled x rows (bf16, replicated input in the padded
core-major table layout), then transforms per dst block:
relu((A x)@W1)@W2, staging h2 into the left half of a 128-wide padded
local table (right half is never written; gathered garbage columns are
never read).  One big AllGather (25.7MB, which reaches the high-bandwidth
regime of the interconnect) replicates the padded table, then layer 2
gathers h2 rows and scatter-sums them the same way.

kernel(**inputs) takes the full unsharded inputs and returns the full
[100000, 64] float32 output.
"""
import sys

sys.path.insert(0, "/opt/trn_rl_repo")

import numpy as np

P = 8          # cores
BLK = 128      # dst nodes per aggregation block (PSUM partition dim)
SBS = 6        # blocks per PSUM group (bounded by the 8 PSUM banks)
GRP = 1        # PSUM groups per gather group
GB = GRP * SBS # blocks per gather group
CW = 24576     # gather-chunk rows (int16 index limit: <= 32768)


def _build_partition(src_f, dst_f, n_nodes):
    """Partition/sort/pad edges; build per-core gather+metadata arrays.

    The gather table is the padded core-major layout [P*npad, ch]: core c's
    rows at c*npad.  Chunks are fixed row windows of CW rows so int16
    indices reach every row; both layers share the same index arrays.
    """
    nc_nodes = n_nodes // P
    nb = -(-nc_nodes // BLK)                      # blocks per core
    npad = nb * BLK
    nsb = -(-nb // SBS)
    ngg = -(-nb // GB)                            # gather groups
    nchunk = -(-(npad * P) // CW)

    # per-core sorted edge arrays
    cores = []
    cnt = np.zeros((P, nb, nchunk), np.int64)
    for c in range(P):
        lo, hi = c * nc_nodes, (c + 1) * nc_nodes
        sel = (dst_f >= lo) & (dst_f < hi)
        s = src_f[sel]
        d = dst_f[sel] - lo
        blk = d // BLK
        scs = s // nc_nodes                       # src core
        sl = s % nc_nodes                         # src local row
        tab = scs * npad + sl                     # padded core-major row
        ch = tab // CW
        tabidx = tab - ch * CW
        order = np.lexsort((tabidx, ch, blk))
        d = d[order]
        blk, ch, tabidx = blk[order], ch[order], tabidx[order]
        key = blk * nchunk + ch
        cnt[c] = np.bincount(key, minlength=nb * nchunk).reshape(nb, nchunk)
        cores.append((tabidx, d))

    L = cnt.max(axis=0)                           # [nb, nchunk] slots per group

    # Call layout: within call (g,k), blocks' slot ranges are packed
    # back-to-back (NOT 128-rounded); only the call total rounds up to 128.
    # A 128-slot tile spanning multiple blocks gets one matmul "unit" per
    # spanned block with a masked one-hot.  All of this bookkeeping is
    # identical across cores (L is a cross-core max), only meta values vary.
    calls = {}
    G = 0     # meta cols (one per unit)
    IC = 0    # idx cols (int16, wrapped into 16 partitions)
    for g in range(ngg):
        bs = list(range(g * GB, min((g + 1) * GB, nb)))
        for k in range(nchunk):
            tot = int(sum(L[b, k] for b in bs))
            if tot == 0:
                calls[(g, k)] = None
                continue
            T = -(-tot // BLK)
            num_idxs = T * BLK
            # slot -> block id (-1 for tail padding)
            blk_of = np.full(num_idxs, -1, np.int64)
            s0 = 0
            starts_b = {}
            for b in bs:
                starts_b[b] = s0
                blk_of[s0 : s0 + int(L[b, k])] = b
                s0 += int(L[b, k])
            units = []                             # per tile: [(block, mcol)]
            for t in range(T):
                span = [b for b in bs
                        if starts_b[b] < (t + 1) * BLK
                        and starts_b[b] + int(L[b, k]) > t * BLK]
                if not span:
                    span = [bs[-1]]
                us = []
                for b in span:
                    us.append((b, G))
                    G += 1
                units.append(us)
            calls[(g, k)] = dict(
                T=T, num_idxs=num_idxs, idx_off=IC, units=units,
                blk_of=blk_of, starts_b=starts_b,
            )
            IC += num_idxs // 16

    # per-core arrays
    idx_alls, mdsts = [], []
    for c in range(P):
        tabidx, d = cores[c]
        starts = np.zeros(nb * nchunk + 1, np.int64)
        np.cumsum(cnt[c].reshape(-1), out=starts[1:])
        idx_all = np.zeros((128, IC), np.int16)
        mdst = np.full((128, G), -1.0, np.float32)
        for g in range(ngg):
            bs = list(range(g * GB, min((g + 1) * GB, nb)))
            for k in range(nchunk):
                call = calls[(g, k)]
                if call is None:
                    continue
                ni = call["num_idxs"]
                iv = np.zeros(ni, np.int16)
                dv = np.full(ni, -1.0, np.float32)
                for b in bs:
                    e0 = starts[b * nchunk + k]
                    n = int(cnt[c, b, k])
                    if n == 0:
                        continue
                    s0 = call["starts_b"][b]
                    iv[s0 : s0 + n] = tabidx[e0 : e0 + n].astype(np.int16)
                    dv[s0 : s0 + n] = (d[e0 : e0 + n] % BLK).astype(np.float32)
                blk_of = call["blk_of"]
                for t, us in enumerate(call["units"]):
                    sl = slice(t * BLK, (t + 1) * BLK)
                    for b, mcol in us:
                        mask = blk_of[sl] == b
                        mdst[:, mcol] = np.where(mask, dv[sl], -1.0)
                wrapped = iv.reshape(-1, 16).T     # [16, cols]
                c0 = call["idx_off"]
                idx_all[:, c0 : c0 + wrapped.shape[1]] = np.tile(wrapped, (8, 1))
        idx_alls.append(idx_all)
        mdsts.append(mdst)

    return dict(
        nc_nodes=nc_nodes, nb=nb, npad=npad, nchunk=nchunk,
        nsb=nsb, ngg=ngg, calls=calls, G=G, IC=IC,
        idx_alls=idx_alls, mdsts=mdsts,
    )


def _trace(nc, tile, mybir, bk, in_ch, hid, out_ch, has_b1, has_b2):
    """Emit the Tile program for one core (SPMD: same program, per-core data)."""
    import os

    phases = os.environ.get("GCN_PHASES", "BD")
    f32 = mybir.dt.float32
    bf16 = mybir.dt.bfloat16
    i16 = mybir.dt.int16
    nb, nchunk, nsb = bk["nb"], bk["nchunk"], bk["nsb"]
    ngg = bk["ngg"]
    npad = bk["npad"]
    RG = [list(range(P))]

    # --- I/O ---
    x_d = nc.dram_tensor("x", [npad, in_ch], bf16, kind="ExternalInput")
    xq_d = nc.dram_tensor("xq", [npad * P, in_ch], bf16, kind="ExternalInput")
    w1_d = nc.dram_tensor("w1", [in_ch, hid], bf16, kind="ExternalInput")
    w2_d = nc.dram_tensor("w2", [hid, out_ch], bf16, kind="ExternalInput")
    iota_d = nc.dram_tensor("iota", [128, 128], bf16, kind="ExternalInput")
    dsc_d = nc.dram_tensor("dsc", [128, nb], f32, kind="ExternalInput")
    ident_d = nc.dram_tensor("ident", [128, 128], bf16, kind="ExternalInput")
    idx_d = nc.dram_tensor("idx", [128, bk["IC"]], i16, kind="ExternalInput")
    mdst_d = nc.dram_tensor("mdst", [128, bk["G"]], f32, kind="ExternalInput")
    if has_b1:
        b1_d = nc.dram_tensor("b1bc", [128, hid], f32, kind="ExternalInput")
    if has_b2:
        b2_d = nc.dram_tensor("b2bc", [128, out_ch], f32, kind="ExternalInput")
    out_d = nc.dram_tensor("out", [npad, out_ch], f32, kind="ExternalOutput")

    # --- internal DRAM ---
    # h2 staging: 64 payload cols inside 128-wide rows keeps gather rows
    # 256B-aligned; the right half is never written and never read
    t2cp = nc.dram_tensor("t2cp", [npad, in_ch], bf16)
    table2p = nc.dram_tensor("table2p", [npad * P, in_ch], bf16,
                             addr_space="Shared")

    with tile.TileContext(nc) as tc:
        with (
            tc.tile_pool(name="const", bufs=1) as cpool,
            tc.tile_pool(name="xt", bufs=4) as xtpool,
            tc.tile_pool(name="hstage", bufs=2) as hpool,
            tc.tile_pool(name="msgs", bufs=10) as mpool,
            tc.tile_pool(name="st", bufs=24) as stpool,
            tc.tile_pool(name="hloc", bufs=8) as hlpool,
        ):
            iota_sb = cpool.tile([128, 128], bf16, tag="iota")
            nc.sync.dma_start(iota_sb[:], iota_d[:])
            dsc_sb = cpool.tile([128, nb], f32, tag="dsc")
            nc.sync.dma_start(dsc_sb[:], dsc_d[:])
            ident_sb = cpool.tile([128, 128], bf16, tag="ident")
            nc.sync.dma_start(ident_sb[:], ident_d[:])
            w1_sb = cpool.tile([in_ch, hid], bf16, tag="w1")
            nc.sync.dma_start(w1_sb[:], w1_d[:])
            w2_sb = cpool.tile([hid, out_ch], bf16, tag="w2")
            nc.sync.dma_start(w2_sb[:], w2_d[:])
            idx_sb = cpool.tile([128, bk["IC"]], i16, tag="idx")
            nc.sync.dma_start(idx_sb[:], idx_d[:])
            mdst_sb = cpool.tile([128, bk["G"]], f32, tag="mdst")
            nc.sync.dma_start(mdst_sb[:], mdst_d[:])
            if has_b1:
                b1_sb = cpool.tile([128, hid], f32, tag="b1")
                nc.sync.dma_start(b1_sb[:], b1_d[:])
            if has_b2:
                b2_sb = cpool.tile([128, out_ch], f32, tag="b2")
                nc.sync.dma_start(b2_sb[:], b2_d[:])

            ag_emitted = [False]

            def emit_ag():
                # one big AllGather: 25.7MB reaches the high-bw regime,
                # cheaper than any chunked schedule (and a collective
                # blocks the issuing engine for its full duration, so
                # total collective time is what matters)
                if ag_emitted[0]:
                    return
                ag_emitted[0] = True
                nc.gpsimd.collective_compute(
                    "AllGather", mybir.AluOpType.bypass, replica_groups=RG,
                    ins=[t2cp[:]], outs=[table2p[:]],
                )

            def aggregate(table_aps, diag_ap, ag_fn, width, msg_w, epilogue,
                          psname, dt, agg_bufs=8):
                """Chunk-major scatter-sum of table[src] into dst blocks.
                Self-loops enter as a constant-identity unit on sequentially
                loaded local rows.  msg_w = gathered row width (>= width;
                extra columns are alignment garbage and are never read)."""
                ngg_lim = min(ngg, int(os.environ.get("GCN_NSB_LIMIT", "9999")))
                with tc.tile_pool(
                    name=psname, bufs=agg_bufs, space="PSUM"
                ) as apool:
                    for g in range(ngg_lim):
                        gbs = list(range(g * GB, min((g + 1) * GB, nb)))
                        nt = {b: 1 for b in gbs}       # +1: diagonal unit
                        for k in range(nchunk):
                            call = bk["calls"][(g, k)]
                            if call is None:
                                continue
                            for us in call["units"]:
                                for b, _ in us:
                                    nt[b] += 1
                        # gathers once per gather group (all chunks)
                        ms = {}
                        for k in range(nchunk):
                            call = bk["calls"][(g, k)]
                            if call is None:
                                continue
                            if ag_fn is not None:
                                ag_fn()
                            T = call["T"]
                            m = mpool.tile([128, T, msg_w], dt, tag="msgs")
                            ms[k] = m
                            c0 = call["idx_off"]
                            nc.gpsimd.dma_gather(
                                m[:],
                                table_aps[k],
                                idx_sb[:, c0 : c0 + call["num_idxs"] // 16],
                                num_idxs=call["num_idxs"],
                                num_idxs_reg=call["num_idxs"],
                                elem_size=msg_w,
                                single_packet=False,
                            )
                        # PSUM accumulation + epilogue per SBS subgroup
                        for sub in range(GRP):
                            bs = gbs[sub * SBS : (sub + 1) * SBS]
                            if not bs:
                                continue
                            sgi = g * GRP + sub
                            bset = set(bs)
                            pss = {}
                            done = {b: 0 for b in bs}
                            for b in bs:
                                hloc = hlpool.tile(
                                    [128, width], dt, tag="hloc"
                                )
                                nc.sync.dma_start(hloc[:], diag_ap(b))
                                pss[b] = apool.tile(
                                    [128, width], f32, tag="agg",
                                    name=f"agg{b}",
                                )
                                nc.tensor.matmul(
                                    pss[b][:], ident_sb[:], hloc[:],
                                    start=True, stop=(nt[b] == 1),
                                )
                                done[b] = 1
                            for k in range(nchunk):
                                call = bk["calls"][(g, k)]
                                if call is None:
                                    continue
                                for t, us in enumerate(call["units"]):
                                    for b, mcol in us:
                                        if b not in bset:
                                            continue
                                        st = stpool.tile(
                                            [128, 128], dt, tag="st"
                                        )
                                        nc.vector.tensor_scalar(
                                            st[:], iota_sb[:],
                                            mdst_sb[:, mcol : mcol + 1],
                                            None,
                                            op0=mybir.AluOpType.is_equal,
                                        )
                                        nc.tensor.matmul(
                                            pss[b][:], st[:],
                                            ms[k][:, t, 0:width],
                                            start=(done[b] == 0),
                                            stop=(done[b] == nt[b] - 1),
                                        )
                                        done[b] += 1
                            for i, b in enumerate(bs):
                                epilogue(sgi, i, pss[b])

            # ---- Phase B: layer-1 aggregation of dinv-scaled x, then the
            # whole transform chain relu((.)@W1)@W2 per dst block in the
            # epilogue (A_hat(x@W1) == (A_hat x)@W1), staging h2 into t2cp.
            rstage = [None]

            def make_epi1(tp2pool):
                def epi1(g, i, ps):
                    b = g * SBS + i
                    bs = min(SBS, nb - g * SBS)
                    if rstage[0] is None:
                        rstage[0] = hpool.tile(
                            [128, bs, out_ch], bf16, tag="hs", name="h2s"
                        )
                    hs = rstage[0]
                    a_sb = xtpool.tile([128, in_ch], bf16, tag="a_sb")
                    nc.scalar.mul(a_sb[:], ps[:], dsc_sb[:, b : b + 1])
                    at_ps = tp2pool.tile([128, 128], bf16, tag="tp", bufs=1)
                    nc.tensor.transpose(at_ps[:], a_sb[:], ident_sb[:])
                    at_sb = xtpool.tile([128, 128], bf16, tag="at_sb")
                    nc.scalar.copy(at_sb[:], at_ps[:])
                    h_ps = tp2pool.tile([128, hid], f32, tag="tpf", bufs=1)
                    nc.tensor.matmul(
                        h_ps[:], at_sb[:], w1_sb[:], start=True, stop=True
                    )
                    r_sb = xtpool.tile([128, hid], bf16, tag="r_sb")
                    if has_b1:
                        nc.vector.tensor_tensor(
                            r_sb[:], h_ps[:], b1_sb[:], mybir.AluOpType.add
                        )
                        nc.scalar.activation(
                            r_sb[:], r_sb[:],
                            mybir.ActivationFunctionType.Relu,
                        )
                    else:
                        nc.scalar.activation(
                            r_sb[:], h_ps[:],
                            mybir.ActivationFunctionType.Relu,
                        )
                    # h2 = relu(...) @ W2 inline, scaled by dinv[src-to-be]
                    rt_ps = tp2pool.tile([128, 128], bf16, tag="tp", bufs=1)
                    nc.tensor.transpose(rt_ps[:], r_sb[:], ident_sb[:])
                    rt_sb = xtpool.tile([128, 128], bf16, tag="at_sb")
                    nc.scalar.copy(rt_sb[:], rt_ps[:])
                    h2_ps = tp2pool.tile([128, out_ch], f32, tag="tpf", bufs=1)
                    nc.tensor.matmul(
                        h2_ps[:], rt_sb[:], w2_sb[:], start=True, stop=True
                    )
                    nc.scalar.mul(hs[:, i, :], h2_ps[:], dsc_sb[:, b : b + 1])
                    if i == bs - 1:
                        r0 = g * SBS * BLK
                        nc.sync.dma_start(
                            t2cp[r0 : r0 + bs * BLK, 0:out_ch].rearrange(
                                "(nb p) c -> p nb c", p=BLK
                            ),
                            hs[:, :, :],
                        )
                        rstage[0] = None
                        if g == nsb - 1:
                            emit_ag()


                return epi1

            if "B" in phases:
                xq_aps = [
                    xq_d[k * CW : min((k + 1) * CW, npad * P), :]
                    for k in range(nchunk)
                ]
                with tc.tile_pool(name="tpB", bufs=1, space="PSUM") as tp2pool:
                    aggregate(
                        xq_aps,
                        lambda b: x_d[b * BLK : (b + 1) * BLK, :],
                        None, in_ch, in_ch, make_epi1(tp2pool), "aggB",
                        bf16, agg_bufs=6,
                    )

            # ---- Phase D: layer-2 aggregation -> out ----
            ostage = [None]

            def epi2(g, i, ps):
                b = g * SBS + i
                bs = min(SBS, nb - g * SBS)
                if ostage[0] is None:
                    ostage[0] = hpool.tile(
                        [128, bs, out_ch], f32, tag="os", name="o2s"
                    )
                os_ = ostage[0]
                nc.scalar.mul(os_[:, i, :], ps[:], dsc_sb[:, b : b + 1])
                if has_b2:
                    nc.vector.tensor_tensor(
                        os_[:, i, :], os_[:, i, :], b2_sb[:],
                        mybir.AluOpType.add,
                    )
                if i == bs - 1:
                    r0 = g * SBS * BLK
                    nc.sync.dma_start(
                        out_d[r0 : r0 + bs * BLK, :].rearrange(
                            "(nb p) c -> p nb c", p=BLK
                        ),
                        os_[:, :, :],
                    )
                    ostage[0] = None

            if "D" in phases:
                table_aps = [
                    table2p[k * CW : min((k + 1) * CW, npad * P), :]
                    for k in range(nchunk)
                ]
                aggregate(
                    table_aps,
                    lambda b: t2cp[b * BLK : (b + 1) * BLK, 0:out_ch],
                    emit_ag, out_ch, in_ch, epi2, "aggD", bf16,
                )


def _prepare(x, edge_index, W1, b1, W2, b2):
    """Host preprocessing + trace + compile. Returns (nc, bk, in_maps)."""
    import concourse.bacc as bacc
    import concourse.mybir as mybir
    from concourse import tile

    bf16np = mybir.dt.np(mybir.dt.bfloat16)

    x = np.asarray(x, dtype=np.float32)
    edge_index = np.asarray(edge_index)
    W1 = np.asarray(W1, dtype=np.float32)
    b1 = np.asarray(b1, dtype=np.float32)
    W2 = np.asarray(W2, dtype=np.float32)
    b2 = np.asarray(b2, dtype=np.float32)

    n_nodes, in_ch = x.shape
    hid = W1.shape[1]
    out_ch = W2.shape[1]
    assert in_ch == 128 and hid == 128, "transform path assumes 128 channels"

    # --- graph preprocessing (index arithmetic only) ---
    src = edge_index[0].astype(np.int64)
    dst = edge_index[1].astype(np.int64)
    loops = np.arange(n_nodes, dtype=np.int64)
    deg = np.bincount(np.concatenate([dst, loops]), minlength=n_nodes)
    deg = deg.astype(np.float32)
    dinv = np.where(deg > 0, 1.0 / np.sqrt(deg), 0.0).astype(np.float32)

    # self-loops are handled as per-block identity units (sequential loads),
    # not as gathered edges
    bk = _build_partition(src, dst, n_nodes)
    nc_nodes, nb, npad = bk["nc_nodes"], bk["nb"], bk["npad"]

    has_b1 = bool(np.any(b1))
    has_b2 = bool(np.any(b2))

    nc = bacc.Bacc(
        "TRN2", target_bir_lowering=False, debug=False, num_devices=P,
        dynamic_dma_scratch_size=32768,
    )
    _trace(nc, tile, mybir, bk, in_ch, hid, out_ch, has_b1, has_b2)
    nc.compile()

    iota_np = np.tile(np.arange(128, dtype=np.float32), (128, 1)).astype(bf16np)
    ident_np = np.eye(128, dtype=np.float32).astype(bf16np)
    xt_f = dinv[:, None] * x                       # dinv[src]-scaled table

    # xq: padded core-major table layout, replicated to every core (no
    # layer-1 AllGather needed); matches table2p's row addressing.
    xq = np.zeros((npad * P, in_ch), bf16np)
    for c in range(P):
        xq[c * npad : c * npad + nc_nodes] = (
            xt_f[c * nc_nodes : (c + 1) * nc_nodes].astype(bf16np)
        )

    in_maps = []
    for c in range(P):
        xs = np.zeros((npad, in_ch), bf16np)
        xs[:nc_nodes] = xt_f[c * nc_nodes : (c + 1) * nc_nodes].astype(bf16np)
        dd = np.zeros(npad, np.float32)
        dd[:nc_nodes] = dinv[c * nc_nodes : (c + 1) * nc_nodes]
        m = dict(
            x=xs, xq=xq, w1=W1.astype(bf16np), w2=W2.astype(bf16np),
            iota=iota_np, ident=ident_np,
            idx=bk["idx_alls"][c], mdst=bk["mdsts"][c],
            dsc=dd.reshape(nb, BLK).T.copy(),
        )
        if has_b1:
            m["b1bc"] = np.tile(b1[None, :], (128, 1)).astype(np.float32)
        if has_b2:
            m["b2bc"] = np.tile(b2[None, :], (128, 1)).astype(np.float32)
        in_maps.append(m)

    return nc, bk, in_maps


def kernel(x, edge_index, W1, b1, W2, b2):
    from concourse.bass_utils import run_bass_kernel_spmd

    nc, bk, in_maps = _prepare(x, edge_index, W1, b1, W2, b2)
    res = run_bass_kernel_spmd(nc, in_maps, core_ids=list(range(P)))
    out = np.concatenate(
        [res.results[c]["out"][: bk["nc_nodes"]] for c in range(P)], axis=0
    )
    return out.astype(np.float32)
